# revision 1
# baseline (speedup 1.0000x reference)
"""Multi-head attention (B=4, S=2048, H=16, D=64, C=1024) on 8 NeuronCores.

Sharding: core c handles batch b=c//2 and head-half half=c%2 (8 heads = 512
inner dims).  Each core computes q/k/v projections for its half of the heads,
full softmax attention over S=2048, and a partial output projection through
its 512 rows of Wo.  Host upcasts + sums the two bf16 partials per batch and
adds the bias.

Per-core kernel layout (all matmul operands bf16, PSUM accumulation fp32):
  xt    [C=1024, S=2048]   hidden_states[b].T          (host pre-transposed)
  wq/wk/wv [C, I=512]      per-half weight columns
  wo    [I=512, C=1024]    per-half weight rows
  qT,kT [I, S] stored as 4 SBUF tiles [128, 2048]  (head pair per tile)
  v_pad [S, 8*65]          v with a ones column per head (row-sum via matmul)
  scores^T per (pair, qi-chunk, kj-tile): [kj=128, qi=512] via row-tiled
  (K=64) matmul pairs; exp on ScalarE; p@[v|1] accumulated in PSUM over kj.

Default schedule (expmode="pipe") software-pipelines the attention kt-loop:
each step issues [<=2 proj/outproj filler matmuls], scores(kt), exp(kt),
pv(kt-1), so scores(kt+1) is never queued behind pv(kt) on the in-order PE
and ScalarE can stream exps.  Projections and the output projection are
spread as half-width (N=512, 1-PSUM-bank) accumulations one matmul per step.
Softmax normalization broadcasts 1/denom via a K=1 PE outer-product against
ones (gpsimd partition_broadcast head-of-line blocks the strict-FIFO DVE
queue and was removed from this path).  PSUM: scores 2x2 banks + acc 2x1 +
pv 2x1 = 8.

Measured (repeat-loop slope, For_i back-edge floor subtracted): the body is
~500-560us and is bound by per-instruction dispatch/semaphore overhead
(~4k instructions) rather than any single engine; PE matmul streams measure
~107ns per N=512 bf16 matmul (2 cols/cycle) + ~107ns serialized LDWEIGHTS +
~70ns fixed, ScalarE exp is (N+352)/1.2GHz.  The empty-loop floor itself
swings 90-410us run to run, so only interleaved A/B minima are comparable.
"""

import functools

import numpy as np
import ml_dtypes

S = 2048          # sequence length
C = 1024          # query dim
I = 512           # inner dims per core (8 heads x 64)
HC = 8            # heads per core
D = 64            # head dim
NCORES = 8
SCALE = D ** -0.5
CT = C // 128     # 8 c-tiles
IT = I // 128     # 4 i-tiles (head pairs)
ST = S // 128     # 16 s-tiles
NQ = S // 512     # 4 qi chunks
VW = D + 1        # 65: v plus ones column


def _build(repeat=1, phases="dma,proj,attn,outproj", expmode="act2"):
    import contextlib

    import concourse.bacc as bacc
    import concourse.tile as tile
    from concourse import mybir

    f32 = mybir.dt.float32
    bf16 = mybir.dt.bfloat16
    fp16 = mybir.dt.float16
    Exp = mybir.ActivationFunctionType.Exp
    is_pipe = expmode in ("pipe", "pipe_s", "no_pv", "pe_only")
    if expmode == "act2":
        pass  # act schedule + balanced out-store queues

    nc = bacc.Bacc("TRN2", target_bir_lowering=False, debug=False,
                   num_devices=NCORES)

    # All inputs are host-pre-shuffled to partition-major [128, ...] layouts so
    # every DMA reads long contiguous runs per partition.
    xt_d = nc.dram_tensor("xt", [128, CT * S], bf16, kind="ExternalInput").ap()
    wq_d = nc.dram_tensor("wq", [128, CT * I], bf16, kind="ExternalInput").ap()
    wk_d = nc.dram_tensor("wk", [128, CT * I], bf16, kind="ExternalInput").ap()
    wv_d = nc.dram_tensor("wv", [128, CT * I], bf16, kind="ExternalInput").ap()
    wo_d = nc.dram_tensor("wo", [128, IT * C], bf16, kind="ExternalInput").ap()
    # bf16 partials: halves the output DMA; host upcasts and sums
    out_d = nc.dram_tensor("out", [S, C], bf16, kind="ExternalOutput").ap()

    with tile.TileContext(nc) as tc:
        with contextlib.ExitStack() as ctx:
            if repeat > 1:
                ctx.enter_context(tc.For_i(0, repeat, 1))
            const = ctx.enter_context(tc.tile_pool(name="const", bufs=1))
            work = ctx.enter_context(tc.tile_pool(name="work", bufs=4))
            outp = ctx.enter_context(tc.tile_pool(name="outp", bufs=4))
            if is_pipe:
                # scores ping-pong (2x2 banks) + half-width proj/outproj
                # accumulators (2x1) + pv accumulators (2x1) = 8 PSUM banks
                sc_pool = ctx.enter_context(
                    tc.tile_pool(name="sc", bufs=2, space="PSUM"))
                acc_pool = ctx.enter_context(
                    tc.tile_pool(name="acc", bufs=2, space="PSUM"))
            else:
                ps_pool = ctx.enter_context(
                    tc.tile_pool(name="ps", bufs=3, space="PSUM"))
            pv_pool = ctx.enter_context(tc.tile_pool(name="pv", bufs=2, space="PSUM"))
            # ---- load inputs (contiguous per partition) ---------------------
            xt_sb = const.tile([128, CT, S], bf16)
            xt_r = xt_d.rearrange("p (t s) -> p t s", s=S)
            for q4 in range(4):
                nc.sync.dma_start(out=xt_sb[:, q4 * 2:(q4 + 1) * 2, :],
                                  in_=xt_r[:, q4 * 2:(q4 + 1) * 2, :])

            wq_sb = const.tile([128, CT, I], bf16)
            nc.scalar.dma_start(out=wq_sb, in_=wq_d.rearrange("p (t i) -> p t i", i=I))
            wk_sb = const.tile([128, CT, I], bf16)
            nc.scalar.dma_start(out=wk_sb, in_=wk_d.rearrange("p (t i) -> p t i", i=I))
            wv_sb = const.tile([128, CT, I], bf16)
            nc.scalar.dma_start(out=wv_sb, in_=wv_d.rearrange("p (t i) -> p t i", i=I))
            wo_sb = const.tile([128, IT, C], bf16)
            nc.scalar.dma_start(out=wo_sb, in_=wo_d.rearrange("p (t c) -> p t c", c=C))

            ones_sb = const.tile([1, 128], bf16)
            nc.vector.memset(ones_sb, 1.0)

            phs = set(phases.split(","))

            # ---- projections -------------------------------------------------
            qT_sb = const.tile([128, IT, S], bf16)
            kT_sb = const.tile([128, IT, S], bf16)
            v_sb = const.tile([128, ST, HC * VW], bf16)
            # ones columns per head (softmax denominator): contiguous memset of
            # the whole tile — the v copies then overwrite the 64 data columns.
            # (A strided single-element memset here measures ~78us on HW.)
            nc.vector.memset(v_sb, 1.0)

            oT_sb = const.tile([128, IT, S], bf16)

            v_main = v_sb.rearrange("p t (h e) -> p t h e", e=VW)[:, :, :, 0:D]

            if is_pipe:
                # ======== software-pipelined schedule ========================
                # Per step: [<=pace filler PE matmuls], scores(kt), exp(kt),
                # pv(kt-1).  scores(kt+1) is never queued behind pv(kt) on the
                # in-order PE, so ACT streams exps back-to-back instead of the
                # serial scores->exp->pv chain per step.
                def qk_step(it, which, nq2, i16, hold):
                    # ct-outer / h2-inner: consecutive matmuls share lhsT so
                    # walrus elides the second LDWEIGHTS (mm_same 286ns vs
                    # mm_rot 408ns in exp_pe.py); two half-accs stay live
                    w_sb, o_sb = ((wq_sb, qT_sb), (wk_sb, kT_sb))[which]
                    ct, h2 = divmod(i16, 2)
                    if i16 == 0:
                        hold[0] = acc_pool.tile(
                            [128, 512], f32, tag="acc", name="qk_a0")
                        hold[1] = acc_pool.tile(
                            [128, 512], f32, tag="acc", name="qk_a1")
                    nc.tensor.matmul(
                        hold[h2],
                        lhsT=w_sb[:, ct, it * 128:(it + 1) * 128],
                        rhs=xt_sb[:, ct, nq2 * 1024 + h2 * 512:
                                  nq2 * 1024 + (h2 + 1) * 512],
                        start=(ct == 0), stop=(ct == CT - 1))
                    if ct == CT - 1:
                        nc.vector.tensor_copy(
                            out=o_sb[:, it, nq2 * 1024 + h2 * 512:
                                     nq2 * 1024 + (h2 + 1) * 512],
                            in_=hold[h2])

                def qk_fillers(it):
                    fl = []
                    for which in range(2):
                        for nq2 in range(2):
                            hold = {}
                            for i16 in range(2 * CT):
                                fl.append(functools.partial(
                                    qk_step, it, which, nq2, i16, hold))
                    return fl

                def v_burst(st):
                    acc = acc_pool.tile([128, 512], f32, tag="acc", name="v_acc")
                    for ct in range(CT):
                        nc.tensor.matmul(
                            acc,
                            lhsT=xt_sb[:, ct, st * 128:(st + 1) * 128],
                            rhs=wv_sb[:, ct, :],
                            start=(ct == 0), stop=(ct == CT - 1))
                    nc.vector.tensor_copy(
                        out=v_main[:, st],
                        in_=acc.rearrange("p (h d) -> p h d", d=D))

                def op_half(st, h2, it, acc_holder):
                    if it == 0:
                        acc_holder["t"] = acc_pool.tile(
                            [128, 512], f32, tag="acc", name="op_acc")
                    nc.tensor.matmul(
                        acc_holder["t"],
                        lhsT=oT_sb[:, it, st * 128:(st + 1) * 128],
                        rhs=wo_sb[:, it, h2 * 512:(h2 + 1) * 512],
                        start=(it == 0), stop=(it == IT - 1))
                    if it == IT - 1:
                        ob = outp.tile([128, 512], bf16, tag="ob", name="ob")
                        nc.vector.tensor_copy(out=ob, in_=acc_holder["t"])
                        eng = nc.sync if (st + h2) % 2 else nc.scalar
                        eng.dma_start(
                            out=out_d[st * 128:(st + 1) * 128,
                                      h2 * 512:(h2 + 1) * 512],
                            in_=ob)

                def outproj_fillers(nqb):
                    fl = []
                    for st in range(4 * nqb, 4 * nqb + 4):
                        for h2 in range(2):
                            hold = {}
                            for it in range(IT):
                                fl.append(functools.partial(
                                    op_half, st, h2, it, hold))
                    return fl

                do_proj = "proj" in phs
                # no_pv/pe_only never write oT -> outproj would read garbage
                # and trip Tile's read-without-write check; drop it there
                do_op = "outproj" in phs and expmode in ("pipe", "pipe_s")

                # prologue: q/k projections for pair 0 and first 4 v tiles
                if do_proj:
                    for which in range(2):
                        for nq2 in range(2):
                            hold = {}
                            for i16 in range(2 * CT):
                                qk_step(0, which, nq2, i16, hold)
                    for st in range(4):
                        v_burst(st)

                def normalize_dve(hp, qs, oA, oB, hold):
                    # copy numerator+denominator out of PSUM right away: the
                    # pv accumulator slots free after one fast DVE copy, so
                    # the next chunk's pv(0) never waits on the full
                    # normalize chain
                    ocA = work.tile([VW, 512], f32, tag="ocopy", name="ocA")
                    ocB = work.tile([VW, 512], f32, tag="ocopy", name="ocB")
                    nc.vector.tensor_copy(out=ocA, in_=oA)
                    nc.vector.tensor_copy(out=ocB, in_=oB)
                    rA32 = work.tile([1, 512], f32, tag="recip", name="rA32")
                    rB32 = work.tile([1, 512], f32, tag="recip", name="rB32")
                    nc.vector.reciprocal(out=rA32, in_=ocA[D:VW, :])
                    nc.vector.reciprocal(out=rB32, in_=ocB[D:VW, :])
                    rA = work.tile([1, 512], bf16, tag="recipb", name="rA")
                    rB = work.tile([1, 512], bf16, tag="recipb", name="rB")
                    nc.vector.tensor_copy(out=rA, in_=rA32)
                    nc.vector.tensor_copy(out=rB, in_=rB32)
                    hold.update(ocA=ocA, ocB=ocB, rA=rA, rB=rB)

                def normalize_pe(hp, qs, hold):
                    # PE outer-product broadcast (K=1 matmul vs ones) -> DVE
                    # multiply.  Runs one step after normalize_dve so the PE
                    # never queues behind an unready reciprocal, and fillers
                    # start at kt>=3 so acc_pool is free here at kt==2.
                    bcA = acc_pool.tile([64, 512], f32, tag="acc", name="bcA")
                    bcB = acc_pool.tile([64, 512], f32, tag="acc", name="bcB")
                    nc.tensor.matmul(bcA, lhsT=ones_sb[0:1, 0:64],
                                     rhs=hold["rA"], start=True, stop=True)
                    nc.tensor.matmul(bcB, lhsT=ones_sb[0:1, 0:64],
                                     rhs=hold["rB"], start=True, stop=True)
                    nc.vector.tensor_mul(
                        out=oT_sb[0:64, hp, qs], in0=hold["ocA"][0:D, :],
                        in1=bcA)
                    nc.vector.tensor_mul(
                        out=oT_sb[64:128, hp, qs], in0=hold["ocB"][0:D, :],
                        in1=bcB)

                # pending closures: pv lags its step by 1; normalize's DVE
                # part runs with the next step's flush (before the next pv so
                # the pv slot is freed by the oc copies), and its PE tail one
                # step after that
                pend = [None]
                pend_nc = [None]
                pend_nt = [None]
                pend_nt2 = [None]

                def flush_pend():
                    if pend_nt2[0] is not None:
                        pend_nt2[0]()
                    pend_nt2[0], pend_nt[0] = pend_nt[0], None
                    if pend_nc[0] is not None:
                        pend_nc[0]()
                        pend_nc[0] = None
                    if pend[0] is not None:
                        pend[0]()
                        pend[0] = None

                if "attn" in phs:
                    for hp in range(IT):
                        qk_next = qk_fillers(hp + 1) if (
                            do_proj and hp < IT - 1) else []
                        for nq in range(NQ):
                            hA, hB = 2 * hp, 2 * hp + 1
                            qs = slice(nq * 512, (nq + 1) * 512)
                            oA = pv_pool.tile([VW, 512], f32, tag="pv", name="oA")
                            oB = pv_pool.tile([VW, 512], f32, tag="pv", name="oB")
                            # fillers for this chunk
                            fl = []
                            if do_proj and hp == 0 and nq == 0:
                                # all v tiles must exist before this chunk's
                                # own pv steps consume them
                                fl += [functools.partial(v_burst, 4 + j)
                                       for j in range(12)]
                            if qk_next:
                                if hp == 0:
                                    if nq > 0:      # unit-aligned 16/16/32
                                        k = [0, 16, 32, 64]
                                        fl += qk_next[k[nq - 1]:k[nq]]
                                else:
                                    fl += qk_next[nq * 16:(nq + 1) * 16]
                            start_kt = 3
                            if do_op and hp == IT - 1 and nq > 0:
                                fl += outproj_fillers(nq - 1)
                            fi = 0
                            for kt in range(ST):
                                if start_kt <= kt <= ST - 2 and fi < len(fl):
                                    todo = len(fl) - fi
                                    left = ST - 1 - kt
                                    n = -(-todo // left)
                                    for _ in range(n):
                                        fl[fi]()
                                        fi += 1
                                sAB = sc_pool.tile([128, 1024], f32, tag="sc",
                                                   name="sAB")
                                ks = slice(kt * 128, (kt + 1) * 128)
                                nc.tensor.matmul(
                                    sAB[:, 0:512], lhsT=kT_sb[0:64, hp, ks],
                                    rhs=qT_sb[0:64, hp, qs],
                                    start=True, stop=True, tile_position=(0, 0))
                                nc.tensor.matmul(
                                    sAB[:, 512:1024], lhsT=kT_sb[64:128, hp, ks],
                                    rhs=qT_sb[64:128, hp, qs],
                                    start=True, stop=True, tile_position=(64, 0))
                                if expmode == "pe_only":
                                    continue
                                pAB = work.tile([128, 1024], bf16, tag="p",
                                                bufs=8, name="pAB")
                                if expmode == "pipe_s":
                                    stg = work.tile([128, 1024], fp16,
                                                    tag="stg", bufs=4,
                                                    name="stg")
                                    nc.vector.tensor_copy(out=stg, in_=sAB)
                                    nc.scalar.activation(out=pAB, in_=stg,
                                                         func=Exp, scale=SCALE)
                                else:
                                    nc.scalar.activation(out=pAB, in_=sAB,
                                                         func=Exp, scale=SCALE)
                                if expmode == "no_pv":
                                    continue
                                flush_pend()

                                def pv(kt=kt, pAB=pAB, oA=oA, oB=oB, hA=hA,
                                       hB=hB, hp=hp, qs=qs):
                                    nc.tensor.matmul(
                                        oA,
                                        lhsT=v_sb[:, kt, hA * VW:(hA + 1) * VW],
                                        rhs=pAB[:, 0:512],
                                        start=(kt == 0), stop=(kt == ST - 1))
                                    nc.tensor.matmul(
                                        oB,
                                        lhsT=v_sb[:, kt, hB * VW:(hB + 1) * VW],
                                        rhs=pAB[:, 512:1024],
                                        start=(kt == 0), stop=(kt == ST - 1))
                                    if kt == ST - 1:
                                        hold = {}
                                        pend_nc[0] = functools.partial(
                                            normalize_dve, hp, qs, oA, oB,
                                            hold)
                                        pend_nt[0] = functools.partial(
                                            normalize_pe, hp, qs, hold)
                                pend[0] = pv
                    flush_pend()
                    flush_pend()
                    flush_pend()
                    if do_op:
                        for f in outproj_fillers(NQ - 1):
                            f()
                elif do_proj:
                    for it in range(1, IT):
                        for f in qk_fillers(it):
                            f()
                    for st in range(4, ST):
                        v_burst(st)

            # ---- projections (non-pipe path) ---------------------------------
            elif "proj" in phs:
                def proj_qk_acc(it, which, nq):
                    w_sb, o_sb = ((wq_sb, qT_sb), (wk_sb, kT_sb))[which]
                    acc = ps_pool.tile([128, 1024], f32, tag="ps",
                                       name="proj_ps")
                    for ct in range(CT):
                        for h2 in range(2):
                            nc.tensor.matmul(
                                acc[:, h2 * 512:(h2 + 1) * 512],
                                lhsT=w_sb[:, ct, it * 128:(it + 1) * 128],
                                rhs=xt_sb[:, ct, nq * 1024 + h2 * 512:
                                          nq * 1024 + (h2 + 1) * 512],
                                start=(ct == 0), stop=(ct == CT - 1))
                    nc.vector.tensor_copy(
                        out=o_sb[:, it, nq * 1024:(nq + 1) * 1024], in_=acc)

                def proj_qk(it):
                    for which in range(2):
                        for nq in range(S // 1024):
                            proj_qk_acc(it, which, nq)

                v_main = v_sb.rearrange("p t (h e) -> p t h e", e=VW)[:, :, :, 0:D]

                def proj_v(st):
                    acc = ps_pool.tile([128, 512], f32, tag="ps", name="v_ps")
                    for ct in range(CT):
                        nc.tensor.matmul(
                            acc,
                            lhsT=xt_sb[:, ct, st * 128:(st + 1) * 128],
                            rhs=wv_sb[:, ct, :],
                            start=(ct == 0), stop=(ct == CT - 1))
                    nc.vector.tensor_copy(
                        out=v_main[:, st],
                        in_=acc.rearrange("p (h d) -> p h d", d=D))

            if not is_pipe:
                # ---- attention + interleaved output projection ------------------
                # expmode:
                #   act    -- per-step exp on ScalarE straight from PSUM (N=1024)
                #   groupG -- DVE copies scores PSUM->fp16 SBUF staging; one big
                #             exp per G steps (N=G*1024) amortizes the ~352-cycle
                #             ACT instruction overhead; pv reads the batched bf16
                G = int(expmode[5:]) if expmode.startswith("group") else 0

                def attn_chunk(hp, nq, extra=None):
                    hA, hB = 2 * hp, 2 * hp + 1
                    qs = slice(nq * 512, (nq + 1) * 512)
                    oA = pv_pool.tile([VW, 512], f32, tag="pv", name="oA")
                    oB = pv_pool.tile([VW, 512], f32, tag="pv", name="oB")

                    def scores_step(kt, pout):
                        if extra is not None:
                            extra(kt)
                        ks = slice(kt * 128, (kt + 1) * 128)
                        sAB = ps_pool.tile([128, 1024], f32, tag="ps", name="sAB")
                        nc.tensor.matmul(
                            sAB[:, 0:512],
                            lhsT=kT_sb[0:64, hp, ks], rhs=qT_sb[0:64, hp, qs],
                            start=True, stop=True, tile_position=(0, 0))
                        nc.tensor.matmul(
                            sAB[:, 512:1024],
                            lhsT=kT_sb[64:128, hp, ks], rhs=qT_sb[64:128, hp, qs],
                            start=True, stop=True, tile_position=(64, 0))
                        if pout is None:
                            pAB = work.tile([128, 1024], bf16, tag="p", bufs=8,
                                            name="pAB")
                            if expmode == "skel":
                                nc.vector.tensor_copy(out=pAB, in_=sAB)
                            elif expmode == "stage1":
                                stg = work.tile([128, 1024], fp16, tag="stg",
                                                bufs=4, name="stg")
                                nc.vector.tensor_copy(out=stg, in_=sAB)
                                nc.scalar.activation(out=pAB, in_=stg, func=Exp,
                                                     scale=SCALE)
                            else:
                                nc.scalar.activation(out=pAB, in_=sAB, func=Exp,
                                                     scale=SCALE)
                            return pAB
                        nc.vector.tensor_copy(out=pout, in_=sAB)
                        return None

                    def pv_step(kt, pA, pB):
                        nc.tensor.matmul(
                            oA, lhsT=v_sb[:, kt, hA * VW:(hA + 1) * VW], rhs=pA,
                            start=(kt == 0), stop=(kt == ST - 1))
                        nc.tensor.matmul(
                            oB, lhsT=v_sb[:, kt, hB * VW:(hB + 1) * VW], rhs=pB,
                            start=(kt == 0), stop=(kt == ST - 1))

                    if G == 0:
                        for kt in range(ST):
                            pAB = scores_step(kt, None)
                            pv_step(kt, pAB[:, 0:512], pAB[:, 512:1024])
                    else:
                        for g in range(ST // G):
                            stg = work.tile([128, G, 1024], fp16, tag="stg",
                                            bufs=2, name="stg")
                            pAB = work.tile([128, G, 1024], bf16, tag="p", bufs=2,
                                            name="pAB")
                            for j in range(G):
                                scores_step(g * G + j, stg[:, j])
                            nc.scalar.activation(out=pAB, in_=stg, func=Exp,
                                                 scale=SCALE)
                            for j in range(G):
                                pv_step(g * G + j, pAB[:, j, 0:512],
                                        pAB[:, j, 512:1024])
                    # normalize: recip of the denominator row, broadcast across the
                    # 64 head dims via a K=1 matmul, multiply on VectorE
                    # normalize: reciprocal of the denominator row (DVE), broadcast
                    # across partitions on the idle GpSimd engine, multiply on DVE
                    rA = work.tile([1, 512], mybir.dt.float32, tag="recip", name="rA")
                    rB = work.tile([1, 512], mybir.dt.float32, tag="recip", name="rB")
                    nc.vector.reciprocal(out=rA, in_=oA[D:VW, :])
                    nc.vector.reciprocal(out=rB, in_=oB[D:VW, :])
                    bcA = work.tile([64, 512], mybir.dt.float32, tag="bcs", name="bcA")
                    bcB = work.tile([64, 512], mybir.dt.float32, tag="bcs", name="bcB")
                    nc.gpsimd.partition_broadcast(bcA, rA)
                    nc.gpsimd.partition_broadcast(bcB, rB)
                    nc.vector.tensor_mul(
                        out=oT_sb[0:64, hp, qs], in0=oA[0:D, :], in1=bcA)
                    nc.vector.tensor_mul(
                        out=oT_sb[64:128, hp, qs], in0=oB[0:D, :], in1=bcB)

                def outproj_tile(st):
                    acc = ps_pool.tile([128, 1024], f32, tag="ps", name="out_ps")
                    for it in range(IT):
                        for h2 in range(2):
                            nc.tensor.matmul(
                                acc[:, h2 * 512:(h2 + 1) * 512],
                                lhsT=oT_sb[:, it, st * 128:(st + 1) * 128],
                                rhs=wo_sb[:, it, h2 * 512:(h2 + 1) * 512],
                                start=(it == 0), stop=(it == IT - 1))
                    ob = outp.tile([128, 1024], bf16, tag="ob", name="ob")
                    nc.vector.tensor_copy(out=ob, in_=acc)
                    if expmode == "act4":
                        # SWDGE ring: both HWDGE rings carry only input
                        # loads, so next iteration's xt/weight reloads are
                        # never queued behind output stores
                        eng = nc.gpsimd
                    elif expmode == "act2" and st % 2 == 0:
                        eng = nc.scalar
                    else:
                        eng = nc.sync
                    eng.dma_start(out=out_d[st * 128:(st + 1) * 128, :], in_=ob)

                if "attn" in phs:
                    # pair-major chunk order; qk projections for the next pair and
                    # v tiles are woven into chunk kt-steps so PE slack under the
                    # ACT-bound exp stream absorbs them.
                    if "proj" in phs:
                        proj_qk(0)
                        for st in range(4):
                            proj_v(st)

                    def make_extra(hp, nq):
                        if "proj" not in phs:
                            return None
                        def extra(kt):
                            if hp == 0 and nq == 0 and 4 + kt < ST:
                                proj_v(4 + kt)
                            if hp < IT - 1 and nq in (1, 2) and kt in (3, 11):
                                acc_idx = (nq - 1) * 2 + (0 if kt == 3 else 1)
                                proj_qk_acc(hp + 1, acc_idx // 2, acc_idx % 2)
                        return extra

                    for hp in range(IT):
                        for nq in range(NQ):
                            attn_chunk(hp, nq, make_extra(hp, nq))
                            if hp == IT - 1 and "outproj" in phs:
                                for st in range(4 * nq, 4 * (nq + 1)):
                                    outproj_tile(st)
                else:
                    if "proj" in phs:
                        for it in range(IT):
                            proj_qk(it)
                        for st in range(ST):
                            proj_v(st)
                    if "outproj" in phs:
                        for st in range(ST):
                            outproj_tile(st)

    nc.compile()
    return nc


@functools.lru_cache(maxsize=8)
def _built(repeat=1, phases="dma,proj,attn,outproj", expmode="act2"):
    return _build(repeat, phases, expmode)


def _pm(a):
    """[T*128, F] -> partition-major [128, T*F] (bf16)."""
    T = a.shape[0] // 128
    return np.ascontiguousarray(
        a.reshape(T, 128, a.shape[1]).swapaxes(0, 1).reshape(128, -1)
    ).astype(ml_dtypes.bfloat16)


def _in_maps(hidden_states, Wq, Wk, Wv, Wo):
    maps = []
    for c in range(NCORES):
        b, half = divmod(c, 2)
        sl = slice(half * I, (half + 1) * I)
        maps.append({
            "xt": _pm(np.ascontiguousarray(hidden_states[b].T)),
            "wq": _pm(Wq[:, sl]),
            "wk": _pm(Wk[:, sl]),
            "wv": _pm(Wv[:, sl]),
            "wo": _pm(Wo[sl, :]),
        })
    return maps


@functools.lru_cache(maxsize=1)
def _runner():
    """Compile the SPMD program once and return a function
    maps -> list of per-core output dicts."""
    import jax
    from jax.sharding import Mesh, PartitionSpec, NamedSharding
    from jax.experimental.shard_map import shard_map

    import concourse.mybir as mybir
    from concourse.bass2jax import (
        _bass_exec_p, install_neuronx_cc_hook, partition_id_tensor)

    nc = _built()
    install_neuronx_cc_hook()
    partition_name = nc.partition_id_tensor.name if nc.partition_id_tensor else None

    in_names, out_names, out_avals, zero_outs = [], [], [], []
    for alloc in nc.m.functions[0].allocations:
        if not isinstance(alloc, mybir.MemoryLocationSet):
            continue
        name = alloc.memorylocations[0].name
        if alloc.kind == "ExternalInput":
            if name != partition_name:
                in_names.append(name)
        elif alloc.kind == "ExternalOutput":
            out_names.append(name)
            shape = tuple(alloc.tensor_shape)
            dtype = mybir.dt.np(alloc.dtype)
            out_avals.append(jax.core.ShapedArray(shape, dtype))
            zero_outs.append(np.zeros(shape, dtype))
    n_params = len(in_names)
    all_in_names = in_names + out_names
    if partition_name is not None:
        all_in_names = all_in_names + [partition_name]

    def _body(*args):
        operands = list(args)
        if partition_name is not None:
            operands.append(partition_id_tensor())
        return tuple(_bass_exec_p.bind(
            *operands,
            out_avals=tuple(out_avals),
            in_names=tuple(all_in_names),
            out_names=tuple(out_names),
            lowering_input_output_aliases=(),
            sim_require_finite=True,
            sim_require_nnan=True,
            nc=nc,
        ))

    devices = jax.devices()[:NCORES]
    mesh = Mesh(np.asarray(devices), ("core",))
    in_specs = (PartitionSpec("core"),) * (n_params + len(out_names))
    out_specs = (PartitionSpec("core"),) * len(out_names)
    sharded = jax.jit(
        shard_map(_body, mesh=mesh, in_specs=in_specs, out_specs=out_specs,
                  check_rep=False),
        keep_unused=True,
    )
    sharding = NamedSharding(mesh, PartitionSpec("core"))
    dev_zero = [jax.device_put(
        np.zeros((NCORES * z.shape[0], *z.shape[1:]), z.dtype), sharding)
        for z in zero_outs]

    def run(maps):
        concat_in = [np.concatenate([np.asarray(maps[c][n]) for c in range(NCORES)],
                                    axis=0) for n in in_names]
        dev_in = [jax.device_put(a, sharding) for a in concat_in]
        out_arrs = sharded(*dev_in, *dev_zero)
        return [
            {n: np.asarray(out_arrs[i]).reshape(NCORES, *out_avals[i].shape)[c]
             for i, n in enumerate(out_names)}
            for c in range(NCORES)
        ]

    return run


def kernel(hidden_states, Wq, Wk, Wv, Wo, bo):
    maps = _in_maps(np.asarray(hidden_states), np.asarray(Wq), np.asarray(Wk),
                    np.asarray(Wv), np.asarray(Wo))
    results = _runner()(maps)
    B = hidden_states.shape[0]
    out = np.empty((B, S, C), np.float32)
    for b in range(B):
        out[b] = (results[2 * b]["out"].astype(np.float32)
                  + results[2 * b + 1]["out"].astype(np.float32))
    out += np.asarray(bo, np.float32)
    return out

